# revision 16
# baseline (speedup 1.0000x reference)
"""Trainium2 Bass kernel for nn_DecoupledSTBlock (B=4,T=24,N=207,D=128).

Sharding: 8 cores = 4 batches x 2 t-halves. Core k handles b=k//2, th=k%2.
 - temporal stream: full batch b (207 seqs of len 24), duplicated across the
   two cores of a batch pair.
 - spatial stream:  t-half (12 seqs of len 207 over nodes), exclusive.
 - fusion + final LN: t-half, exclusive; output slice (12,207,128) per core.

Mamba selective scan runs on the DVE tensor_tensor_scan primitive with layout
[d_inner(part) x (n_state, seq, t)(free)]; decay dA = exp(-(n+1)*dt) built via
multiplicative power chains exploiting A[d,n] = -(n+1) (verified at runtime).
"""
import sys
import numpy as np

sys.path.insert(0, '/opt/trn_rl_repo')

B, T, N, D = 4, 24, 207, 128
DI, DS, DC, DTR = 256, 16, 4, 8
EPS = 1e-5
NP2 = 256           # padded N for the spatial stream
TH = T // 2         # 12
TOKT = N * T        # 4968 temporal tokens per core
TOKF = TH * N       # 2484 fusion tokens per core
GT = 9              # temporal seq-group size (207 = 23*9)
NGT = N // GT       # 23 groups
FT = GT * T         # 216 tokens per temporal group

_CACHE = {}


def _build():
    import concourse.bass as bass
    import concourse.bacc as bacc
    import concourse.tile as tile
    import concourse.mybir as mybir
    from concourse.masks import make_identity

    F32 = mybir.dt.float32
    AL = mybir.AluOpType
    AF = mybir.ActivationFunctionType
    AX = mybir.AxisListType

    nc = bacc.Bacc('TRN2', target_bir_lowering=False, debug=False)

    def din(name, shape):
        return nc.dram_tensor(name, shape, F32, kind='ExternalInput')

    # ---- dram inputs ----
    x_nt = din('x_nt', (TOKT, D))            # x[b] as (n,t,d) flattened
    x_sp = din('x_sp', (TH, NP2, D))         # x[b, t-half] padded to NP2 nodes
    thf = din('thf', (128, 1))               # 0.0 / 1.0 t-half selector
    adjT = din('adjT', (NP2, NP2))           # adj.T zero-padded

    def mamba_ins(p):
        return dict(
            inWT=din(p + 'inWT', (D, 2 * DI)),      # (g-folded in_proj_w).T
            inB=din(p + 'inB', (2 * DI, 1)),        # in_proj_w @ norm_b
            convW=din(p + 'convW', (DI, DC)),
            convB=din(p + 'convB', (DI, 1)),
            xprWT=din(p + 'xprWT', (DI, DTR + 2 * DS)),
            dtWT=din(p + 'dtWT', (DTR, DI)),
            dtB=din(p + 'dtB', (DI, 1)),
            Dv=din(p + 'Dv', (DI, 1)),
            outWT=din(p + 'outWT', (DI, D)),
        )
    tmw = mamba_ins('tm_')
    smw = mamba_ins('sm_')

    msgWr = din('msgWr', (D, D))             # msg_w.T
    U1r = din('U1r', (D, D))                 # upd_w[:, :D].T
    U2r = din('U2r', (D, D))                 # upd_w[:, D:].T
    updb = din('updb', (1, D))               # upd_b + upd_w[:,D:] @ msg_b
    gng = din('gng', (1, D))                 # gp norm_g
    gnb = din('gnb', (1, D))                 # gp norm_b
    W1T = din('W1T', (2 * D, D))             # fusion w1.T
    b1 = din('b1', (D, 1))
    w2d = din('w2d', (D, 1))                 # w2[0]-w2[1]
    b2d = din('b2d', (1, 1))                 # b2[0]-b2[1]
    fng = din('fng', (1, D))
    fnb = din('fnb', (1, D))

    out_t = nc.dram_tensor('out', (TOKF, D), F32, kind='ExternalOutput')

    with tile.TileContext(nc) as tc:
        import contextlib
        ctx = contextlib.ExitStack()
        wp = ctx.enter_context(tc.tile_pool(name='wp', bufs=1))
        pp = ctx.enter_context(tc.tile_pool(name='pp', bufs=1))
        gp_ = ctx.enter_context(tc.tile_pool(name='gp', bufs=2))
        sc = ctx.enter_context(tc.tile_pool(name='sc', bufs=1))
        ps = ctx.enter_context(tc.tile_pool(name='ps', bufs=8, space='PSUM'))

        ident = wp.tile([128, 128], F32)
        make_identity(nc, ident)
        ones_row = wp.tile([1, 128], F32)
        nc.vector.memset(ones_row, 1.0)

        def load_w(t, shape):
            """Load dram weight. Rows>128 become [128, R//128, C] (row-block k
            at [:, k, :])."""
            if shape[0] > 128:
                assert shape[0] % 128 == 0 and len(shape) == 2
                nk = shape[0] // 128
                s = wp.tile([128, nk, shape[1]], F32, tag=t.name)
                nc.sync.dma_start(
                    out=s, in_=t.ap().rearrange('(k p) c -> p k c', p=128))
            else:
                s = wp.tile(list(shape), F32, tag=t.name)
                nc.sync.dma_start(out=s, in_=t.ap())
            return s

        def bcast_row(t, L):
            """(1,L) dram vec -> [128, L] sbuf replicated."""
            s = wp.tile([128, L], F32, tag=t.name + '_bc')
            src = t.ap()
            ap = bass.AP(tensor=src.tensor, offset=src.offset,
                         ap=[[0, 128]] + src.ap[1:])
            nc.sync.dma_start(out=s, in_=ap)
            return s

        # weights in sbuf
        W = {}
        for nm, tens in [('adjT', adjT), ('msgWr', msgWr), ('U1r', U1r),
                         ('U2r', U2r), ('W1T', W1T), ('b1', b1),
                         ('w2d', w2d), ('b2d', b2d), ('thf', thf)]:
            W[nm] = load_w(tens, tens.shape)
        for pfx, mw in [('tm', tmw), ('sm', smw)]:
            for nm, tens in mw.items():
                W[pfx + nm] = load_w(tens, tens.shape)
        W['updb'] = bcast_row(updb, D)
        W['gng'] = bcast_row(gng, D)
        W['gnb'] = bcast_row(gnb, D)
        W['fng'] = bcast_row(fng, D)
        W['fnb'] = bcast_row(fnb, D)
        epsc = wp.tile([128, 1], F32)
        nc.vector.memset(epsc, EPS)

        def ln_norm(dst, src_raw, n_par):
            """dst = (src - mean)/sqrt(var+eps) over free dim D. [p, D] tiles."""
            st = gp_.tile([128, 6], F32, tag='ln_st')
            mv = gp_.tile([128, 2], F32, tag='ln_mv')
            nc.vector.bn_stats(out=st[:n_par], in_=src_raw)
            nc.vector.bn_aggr(out=mv[:n_par], in_=st[:n_par])
            nc.scalar.activation(out=mv[:n_par, 1:2], in_=mv[:n_par, 1:2],
                                 func=AF.Sqrt, bias=epsc[:n_par], scale=1.0)
            nc.vector.reciprocal(out=mv[:n_par, 1:2], in_=mv[:n_par, 1:2])
            nc.vector.tensor_scalar(out=dst, in0=src_raw,
                                    scalar1=mv[:n_par, 0:1],
                                    scalar2=mv[:n_par, 1:2],
                                    op0=AL.subtract, op1=AL.mult)

        def transpose128(dst, src, n_par, n_free):
            """dst[f, p] = src[p, f] via PE. src sbuf [n_par, n_free]."""
            pt = ps.tile([128, 128], F32, tag='tp_ps')
            nc.tensor.transpose(pt[:n_free, :n_par], src, ident[:n_par, :n_par])
            nc.scalar.copy(out=dst, in_=pt[:n_free, :n_par])

        # ============== temporal stream ==============
        xtT = pp.tile([128, TOKT], F32, tag='xtT')   # temporal stream output^T

        def mamba_group(pfx, xlnT_ap, xT_res_ap, dstT_ap, gtok, nseq, Lpad, Lval,
                        conv_pad_tag):
            """One seq-group through the mamba inner. gtok = nseq*Lpad tokens.
            xlnT_ap: [128, gtok] LN'd input^T (pre-gamma-folded).
            xT_res_ap: [128, gtok] raw input^T for residual.
            dstT_ap: [128, gtok] output^T destination.
            Lval: valid steps per seq (scan/compute width per seq)."""
            # in_proj -> psum blocks [128, gtok] x4; copy out immediately
            xinp = sc.tile([128, 2, nseq, DC - 1 + Lpad], F32, tag=conv_pad_tag)
            zt = gp_.tile([128, 2, gtok], F32, tag='m_z')
            for eb in range(4):
                pt = ps.tile([128, gtok], F32, tag='mm_xz')
                nc.tensor.matmul(pt, W[pfx + 'inWT'][:, eb * 128:(eb + 1) * 128],
                                 xlnT_ap, start=True, stop=True)
                bias_ap = W[pfx + 'inB'][:, eb, :]
                if eb < 2:
                    nc.vector.memset(xinp[:, eb, :, 0:DC - 1], 0.0)
                    nc.scalar.activation(out=xinp[:, eb, :, DC - 1:], in_=pt,
                                         func=AF.Identity, bias=bias_ap, scale=1.0)
                else:
                    nc.scalar.activation(out=zt[:, eb - 2], in_=pt,
                                         func=AF.Identity, bias=bias_ap, scale=1.0)
            # conv (depthwise, 4 taps) + silu -> u
            ut = gp_.tile([128, 2, gtok], F32, tag='m_u')
            cacc = gp_.tile([128, 2, gtok], F32, tag='m_cacc')
            for tau in range(2):
                cw = W[pfx + 'convW'][:, tau, :]
                src = xinp[:, tau]
                nc.vector.tensor_scalar_mul(
                    out=cacc[:, tau].rearrange('p (s l) -> p s l', s=nseq),
                    in0=src[:, :, 0:Lpad], scalar1=cw[:, 0:1])
                for j in range(1, DC):
                    nc.vector.scalar_tensor_tensor(
                        out=cacc[:, tau].rearrange('p (s l) -> p s l', s=nseq),
                        in0=src[:, :, j:j + Lpad], scalar=cw[:, j:j + 1],
                        in1=cacc[:, tau].rearrange('p (s l) -> p s l', s=nseq),
                        op0=AL.mult, op1=AL.add)
                nc.scalar.activation(out=ut[:, tau], in_=cacc[:, tau],
                                     func=AF.Silu,
                                     bias=W[pfx + 'convB'][:, tau, :],
                                     scale=1.0)
            # x_proj: [40, gtok]
            xdb_p = ps.tile([128, gtok], F32, tag='mm_xdb')
            for tau in range(2):
                nc.tensor.matmul(xdb_p[:DTR + 2 * DS],
                                 W[pfx + 'xprWT'][:, tau, :],
                                 ut[:, tau], start=(tau == 0), stop=(tau == 1))
            xdb = gp_.tile([128, gtok], F32, tag='m_xdb')
            nc.scalar.copy(out=xdb[:DTR + 2 * DS], in_=xdb_p[:DTR + 2 * DS])
            # dt = softplus(dtWT.T @ dtr + dtB)
            dt = gp_.tile([128, 2, gtok], F32, tag='m_dt')
            for tau in range(2):
                dtp = ps.tile([128, gtok], F32, tag='mm_dtp')
                nc.tensor.matmul(dtp, W[pfx + 'dtWT'][:, tau * 128:(tau + 1) * 128],
                                 xdb[:DTR], start=True, stop=True)
                nc.scalar.activation(out=dt[:, tau], in_=dtp, func=AF.Softplus,
                                     bias=W[pfx + 'dtB'][:, tau, :],
                                     scale=1.0)
            # g = dt * u
            gdt = gp_.tile([128, 2, gtok], F32, tag='m_g')
            nc.vector.tensor_mul(out=gdt, in0=dt, in1=ut)
            # scan per di-tile; B/C rows flattened to partition 0 by DMA,
            # broadcast to 128 partitions via ones-matmul, consumed from PSUM
            nsl = nseq * Lpad
            brow = gp_.tile([1, 2 * DS, nsl], F32, tag='m_brow', bufs=1)
            nc.sync.dma_start(out=brow, in_=xdb[DTR:DTR + 2 * DS])
            yv = gp_.tile([128, 2, gtok], F32, tag='m_y')
            for tau in range(2):
                dA = sc.tile([128, DS, nsl], F32, tag='m_dA')
                Xb = sc.tile([128, DS, nsl], F32, tag='m_X')
                hb = sc.tile([128, DS, nsl], F32, tag='m_h')
                # dA[0] = exp(-dt); zero t=0 of each seq
                nc.scalar.activation(out=dA[:, 0], in_=dt[:, tau],
                                     func=AF.Exp, scale=-1.0)
                nc.vector.memset(
                    dA[:, 0].rearrange('p (s l) -> p s l', s=nseq)[:, :, 0:1], 0.0)
                for n in range(1, DS):
                    nc.vector.tensor_mul(out=dA[:, n], in0=dA[:, n - 1],
                                         in1=dA[:, 0])
                for n in range(DS):
                    pt = ps.tile([128, nsl], F32, tag='mm_bc', name='mm_bcB')
                    nc.tensor.matmul(pt, ones_row, brow[:, n, :],
                                     start=True, stop=True)
                    nc.vector.tensor_mul(out=Xb[:, n], in0=pt,
                                         in1=gdt[:, tau])
                nc.vector.tensor_tensor_scan(
                    out=hb.rearrange('p a b -> p (a b)'),
                    data0=dA.rearrange('p a b -> p (a b)'),
                    data1=Xb.rearrange('p a b -> p (a b)'),
                    initial=0.0, op0=AL.mult, op1=AL.add)
                # y = sum_n h*C ; reuse Xb as product buffer
                for n in range(DS):
                    pt = ps.tile([128, nsl], F32, tag='mm_bc', name='mm_bcC')
                    nc.tensor.matmul(pt, ones_row, brow[:, DS + n, :],
                                     start=True, stop=True)
                    nc.vector.tensor_mul(out=Xb[:, n], in0=pt, in1=hb[:, n])
                nc.vector.tensor_reduce(
                    out=yv[:, tau], in_=Xb.rearrange('p a b -> p b a'),
                    axis=AX.X, op=AL.add)
                # y += u * D
                nc.vector.scalar_tensor_tensor(
                    out=yv[:, tau], in0=ut[:, tau],
                    scalar=W[pfx + 'Dv'][:, tau, :],
                    in1=yv[:, tau], op0=AL.mult, op1=AL.add)
                # y *= silu(z)
                nc.scalar.activation(out=zt[:, tau], in_=zt[:, tau], func=AF.Silu)
                nc.vector.tensor_mul(out=yv[:, tau], in0=yv[:, tau],
                                     in1=zt[:, tau])
            # out_proj + residual
            op = ps.tile([128, gtok], F32, tag='mm_op')
            for tau in range(2):
                nc.tensor.matmul(op, W[pfx + 'outWT'][:, tau, :],
                                 yv[:, tau], start=(tau == 0), stop=(tau == 1))
            nc.vector.scalar_tensor_tensor(out=dstT_ap, in0=op, scalar=1.0,
                                           in1=xT_res_ap, op0=AL.mult, op1=AL.add)

        for g in range(NGT):
            c0 = g * FT
            xTg = gp_.tile([128, FT], F32, tag='t_xTg')
            xlnTg = gp_.tile([128, FT], F32, tag='t_xlnTg')
            for sub in range(2):
                r0 = c0 + sub * 128
                nr = min(128, FT - sub * 128)
                xr = gp_.tile([128, D], F32, tag='t_xr')
                nc.sync.dma_start(out=xr[:nr], in_=x_nt.ap()[r0:r0 + nr])
                xl = gp_.tile([128, D], F32, tag='t_xl')
                ln_norm(xl[:nr], xr[:nr], nr)
                transpose128(xTg[:, sub * 128:sub * 128 + nr], xr[:nr], nr, D)
                transpose128(xlnTg[:, sub * 128:sub * 128 + nr], xl[:nr], nr, D)
            mamba_group('tm', xlnTg, xTg, xtT[:, c0:c0 + FT], FT, GT, T, T,
                        'convp_t')

        # ============== spatial stream ==============
        xspT = pp.tile([128, TH, NP2], F32, tag='xspT')   # spatial mamba out^T (padded n)
        for t2 in range(TH):
            # load raw [NP2, D] (2 tiles), transpose
            xr = [gp_.tile([128, D], F32, tag='s_xr' + str(h), name='s_xr' + str(h))
                  for h in range(2)]
            for h in range(2):
                nc.sync.dma_start(out=xr[h], in_=x_sp.ap()[t2, h * 128:(h + 1) * 128])
            xsT = gp_.tile([128, NP2], F32, tag='s_xsT')
            for h in range(2):
                transpose128(xsT[:, h * 128:(h + 1) * 128], xr[h], 128, D)
            # msg = x @ msgW.T  [tok, d] per half
            msg = gp_.tile([128, 2, D], F32, tag='s_msg')
            for h in range(2):
                pt = ps.tile([128, D], F32, tag='mm_msg')
                nc.tensor.matmul(pt, xsT[:, h * 128:(h + 1) * 128], W['msgWr'],
                                 start=True, stop=True)
                nc.scalar.copy(out=msg[:, h], in_=pt)
            # agg[nb] = sum_m adjT[m, nb].T... out [n-block, d]
            aggT = gp_.tile([128, NP2], F32, tag='s_aggT')
            for nb in range(2):
                pt = ps.tile([128, D], F32, tag='mm_agg')
                for km in range(2):
                    nc.tensor.matmul(pt,
                                     W['adjT'][:, km, nb * 128:(nb + 1) * 128],
                                     msg[:, km], start=(km == 0), stop=(km == 1))
                ag = gp_.tile([128, D], F32, tag='s_ag')
                nc.scalar.copy(out=ag, in_=pt)
                transpose128(aggT[:, nb * 128:(nb + 1) * 128], ag, 128, D)
            # upd + residual + graph LN -> xg [tok, d]
            xgT = gp_.tile([128, NP2], F32, tag='s_xgT')    # LN_graph(xg)^T pre-sm-LN
            xg_td = [gp_.tile([128, D], F32, tag='s_xg' + str(h), name='s_xg' + str(h))
                     for h in range(2)]
            for h in range(2):
                pt = ps.tile([128, D], F32, tag='mm_upd')
                nc.tensor.matmul(pt, xsT[:, h * 128:(h + 1) * 128], W['U1r'],
                                 start=True, stop=False)
                nc.tensor.matmul(pt, aggT[:, h * 128:(h + 1) * 128], W['U2r'],
                                 start=False, stop=True)
                up = gp_.tile([128, D], F32, tag='s_up')
                nc.vector.scalar_tensor_tensor(out=up, in0=pt, scalar=1.0,
                                               in1=xr[h], op0=AL.mult, op1=AL.add)
                nc.vector.tensor_add(out=up, in0=up, in1=W['updb'])
                ln_norm(up, up, 128)
                nc.vector.tensor_mul(out=up, in0=up, in1=W['gng'])
                nc.vector.tensor_add(out=up, in0=up, in1=W['gnb'])
                nc.vector.tensor_copy(out=xg_td[h], in_=up)
            # sm pre-LN (gamma folded into sm inWT)
            xgl = gp_.tile([128, D], F32, tag='s_xgl')
            for h in range(2):
                ln_norm(xgl, xg_td[h], 128)
                transpose128(xgT[:, h * 128:(h + 1) * 128], xgl, 128, D)
            # residual^T for the mamba block = xg^T
            xgrT = gp_.tile([128, NP2], F32, tag='s_xgrT')
            for h in range(2):
                transpose128(xgrT[:, h * 128:(h + 1) * 128], xg_td[h], 128, D)
            mamba_group('sm', xgT, xgrT, xspT[:, t2], NP2, 1, NP2, N, 'convp_s')

        # ============== fusion ==============
        # xt half-select: view xtT cols as (t2, n): col = n*T + t0 + t2
        xtv = xtT.rearrange('p (n t) -> p t n', n=N)   # [128, T, N]
        xsel = pp.tile([128, TOKF], F32, tag='xsel')
        xsel3 = xsel.rearrange('p (a b) -> p a b', a=TH)
        nc.vector.tensor_tensor(out=xsel3, in0=xtv[:, TH:, :],
                                in1=xtv[:, :TH, :], op=AL.subtract)
        nc.vector.scalar_tensor_tensor(
            out=xsel3, in0=xsel3, scalar=W['thf'], in1=xtv[:, :TH, :],
            op0=AL.mult, op1=AL.add)
        # xsp valid view -> contiguous
        xspv = pp.tile([128, TOKF], F32, tag='xspv')
        nc.vector.tensor_copy(
            out=xspv.rearrange('p (a b) -> p a b', a=TH),
            in_=xspT[:, :, 0:N])
        # h1 = gelu(W1 @ comb + b1)
        h1 = pp.tile([128, TOKF], F32, tag='xtT', name='h1')
        nck = (TOKF + 511) // 512
        for ic in range(nck):
            c0 = ic * 512
            cw_ = min(512, TOKF - c0)
            pt = ps.tile([128, 512], F32, tag='mm_h1')
            nc.tensor.matmul(pt[:, :cw_], W['W1T'][:, 0, :], xsel[:, c0:c0 + cw_],
                             start=True, stop=False)
            nc.tensor.matmul(pt[:, :cw_], W['W1T'][:, 1, :], xspv[:, c0:c0 + cw_],
                             start=False, stop=True)
            nc.scalar.activation(out=h1[:, c0:c0 + cw_], in_=pt[:, :cw_],
                                 func=AF.Gelu, bias=W['b1'], scale=1.0)
        # gate logit diff -> sigmoid
        g0r = pp.tile([1, TOKF], F32, tag='xspT', name='g0r')
        for ic in range(nck):
            c0 = ic * 512
            cw_ = min(512, TOKF - c0)
            pt = ps.tile([1, 512], F32, tag='mm_g0')
            nc.tensor.matmul(pt[:, :cw_], W['w2d'], h1[:, c0:c0 + cw_],
                             start=True, stop=True)
            nc.scalar.activation(out=g0r[:, c0:c0 + cw_], in_=pt[:, :cw_],
                                 func=AF.Sigmoid, bias=W['b2d'], scale=1.0)
        # g0 column form [128, 20]
        ntf = (TOKF + 127) // 128
        g0c = gp_.tile([128, ntf], F32, tag='f_g0c')
        for it in range(ntf):
            r0 = it * 128
            nr = min(128, TOKF - r0)
            pt = ps.tile([128, 1], F32, tag='tp_g0')
            nc.tensor.transpose(pt[:nr, 0:1], g0r[:, r0:r0 + nr], ident[0:1, 0:1])
            nc.scalar.copy(out=g0c[:nr, it:it + 1], in_=pt[:nr, 0:1])
        # back to [tok, d], blend, final LN
        for it in range(ntf):
            r0 = it * 128
            nr = min(128, TOKF - r0)
            xtd = gp_.tile([128, D], F32, tag='f_xtd')
            xsd = gp_.tile([128, D], F32, tag='f_xsd')
            transpose128(xtd[:nr], xsel[:, r0:r0 + nr], 128, nr)
            transpose128(xsd[:nr], xspv[:, r0:r0 + nr], 128, nr)
            dd = gp_.tile([128, D], F32, tag='f_dd')
            nc.vector.tensor_sub(out=dd[:nr], in0=xtd[:nr], in1=xsd[:nr])
            fu = gp_.tile([128, D], F32, tag='f_fu')
            nc.vector.scalar_tensor_tensor(out=fu[:nr], in0=dd[:nr],
                                           scalar=g0c[:nr, it:it + 1],
                                           in1=xsd[:nr], op0=AL.mult, op1=AL.add)
            ln_norm(fu[:nr], fu[:nr], nr)
            nc.vector.tensor_mul(out=fu[:nr], in0=fu[:nr], in1=W['fng'][:nr])
            nc.vector.tensor_add(out=fu[:nr], in0=fu[:nr], in1=W['fnb'][:nr])
            nc.sync.dma_start(out=out_t.ap()[r0:r0 + nr], in_=fu[:nr])

        ctx.close()
    nc.compile()
    return nc


def _prep_inputs(x, adj, tm, sm, gp, fp):
    """Host-side weight prep; returns (shared weight map, per-core input maps)."""
    f32 = np.float32

    def mamba_prep(p, w):
        A = -np.exp(np.asarray(w['A_log'], f32))
        expect = -np.broadcast_to(np.arange(1, DS + 1, dtype=f32), (DI, DS))
        assert np.allclose(A, expect, rtol=1e-5, atol=1e-5), \
            'A_log structure changed; scan power-chain invalid'
        g = np.asarray(w['norm_g'], f32)
        b = np.asarray(w['norm_b'], f32)
        iw = np.asarray(w['in_proj_w'], f32)
        return {
            p + 'inWT': np.ascontiguousarray((iw * g[None, :]).T),
            p + 'inB': (iw @ b).reshape(2 * DI, 1),
            p + 'convW': np.asarray(w['conv_w'], f32).reshape(DI, DC),
            p + 'convB': np.asarray(w['conv_b'], f32).reshape(DI, 1),
            p + 'xprWT': np.ascontiguousarray(np.asarray(w['x_proj_w'], f32).T),
            p + 'dtWT': np.ascontiguousarray(np.asarray(w['dt_proj_w'], f32).T),
            p + 'dtB': np.asarray(w['dt_proj_b'], f32).reshape(DI, 1),
            p + 'Dv': np.asarray(w['D'], f32).reshape(DI, 1),
            p + 'outWT': np.ascontiguousarray(np.asarray(w['out_proj_w'], f32).T),
        }

    shared = {}
    shared.update(mamba_prep('tm_', tm))
    shared.update(mamba_prep('sm_', sm))
    adjT = np.zeros((NP2, NP2), f32)
    adjT[:N, :N] = np.asarray(adj, f32).T
    shared['adjT'] = adjT
    uw = np.asarray(gp['upd_w'], f32)
    shared['msgWr'] = np.ascontiguousarray(np.asarray(gp['msg_w'], f32).T)
    shared['U1r'] = np.ascontiguousarray(uw[:, :D].T)
    shared['U2r'] = np.ascontiguousarray(uw[:, D:].T)
    shared['updb'] = (np.asarray(gp['upd_b'], f32)
                      + uw[:, D:] @ np.asarray(gp['msg_b'], f32)).reshape(1, D)
    shared['gng'] = np.asarray(gp['norm_g'], f32).reshape(1, D)
    shared['gnb'] = np.asarray(gp['norm_b'], f32).reshape(1, D)
    w1 = np.asarray(fp['w1'], f32)
    w2 = np.asarray(fp['w2'], f32)
    b2 = np.asarray(fp['b2'], f32)
    shared['W1T'] = np.ascontiguousarray(w1.T)
    shared['b1'] = np.asarray(fp['b1'], f32).reshape(D, 1)
    shared['w2d'] = (w2[0] - w2[1]).reshape(D, 1)
    shared['b2d'] = np.array([[b2[0] - b2[1]]], f32)
    shared['fng'] = np.asarray(fp['norm_g'], f32).reshape(1, D)
    shared['fnb'] = np.asarray(fp['norm_b'], f32).reshape(1, D)

    x = np.asarray(x, f32)
    in_maps = []
    for k in range(8):
        b_, th = k // 2, k % 2
        m = dict(shared)
        m['x_nt'] = np.ascontiguousarray(
            x[b_].transpose(1, 0, 2)).reshape(TOKT, D)
        xsp = np.zeros((TH, NP2, D), f32)
        xsp[:, :N] = x[b_, th * TH:(th + 1) * TH]
        m['x_sp'] = xsp
        m['thf'] = np.full((128, 1), float(th), f32)
        in_maps.append(m)
    return in_maps


def kernel(x, adj, tm, sm, gp, fp):
    from concourse.bass_utils import run_bass_kernel_spmd
    if 'nc' not in _CACHE:
        _CACHE['nc'] = _build()
    nc = _CACHE['nc']
    in_maps = _prep_inputs(x, adj, tm, sm, gp, fp)
    res = run_bass_kernel_spmd(nc, in_maps, core_ids=list(range(8)))
    out = np.empty((B, T, N, D), np.float32)
    for k in range(8):
        b_, th = k // 2, k % 2
        out[b_, th * TH:(th + 1) * TH] = res.results[k]['out'].reshape(TH, N, D)
    return out


# revision 20
# speedup vs baseline: 1.2336x; 1.2336x over previous
"""Trainium2 Bass kernel for nn_DecoupledSTBlock (B=4,T=24,N=207,D=128).

Sharding: 8 cores = 4 batches x 2 t-halves. Core k handles b=k//2, th=k%2.
 - temporal stream: full batch b (207 seqs of len 24), duplicated across the
   two cores of a batch pair.
 - spatial stream:  t-half (12 seqs of len 207 over nodes), exclusive.
 - fusion + final LN: t-half, exclusive; output slice (12,207,128) per core.

Mamba selective scan runs on the DVE tensor_tensor_scan primitive with layout
[d_inner(part) x (n_state, seq, t)(free)]; decay dA = exp(-(n+1)*dt) built via
multiplicative power chains exploiting A[d,n] = -(n+1) (verified at runtime).
"""
import sys
import numpy as np

sys.path.insert(0, '/opt/trn_rl_repo')

B, T, N, D = 4, 24, 207, 128
DI, DS, DC, DTR = 256, 16, 4, 8
EPS = 1e-5
NP2 = 256           # padded N for the spatial stream
TH = T // 2         # 12
TOKT = N * T        # 4968 temporal tokens per core
TOKF = TH * N       # 2484 fusion tokens per core
GT = 9              # temporal seq-group size (207 = 23*9)
NGT = N // GT       # 23 groups
FT = GT * T         # 216 tokens per temporal group

_CACHE = {}


def _build():
    import concourse.bass as bass
    import concourse.bacc as bacc
    import concourse.tile as tile
    import concourse.mybir as mybir
    from concourse.masks import make_identity

    F32 = mybir.dt.float32
    AL = mybir.AluOpType
    AF = mybir.ActivationFunctionType
    AX = mybir.AxisListType

    nc = bacc.Bacc('TRN2', target_bir_lowering=False, debug=False)

    def din(name, shape):
        return nc.dram_tensor(name, shape, F32, kind='ExternalInput')

    # ---- dram inputs ----
    x_nt = din('x_nt', (TOKT, D))            # x[b] as (n,t,d) flattened
    x_sp = din('x_sp', (TH, NP2, D))         # x[b, t-half] padded to NP2 nodes
    thf = din('thf', (128, 1))               # 0.0 / 1.0 t-half selector
    adjT = din('adjT', (NP2, NP2))           # adj.T zero-padded

    def mamba_ins(p):
        return dict(
            inWT=din(p + 'inWT', (D, 2 * DI)),      # (g-folded in_proj_w).T
            inB=din(p + 'inB', (2 * DI, 1)),        # in_proj_w @ norm_b
            convW=din(p + 'convW', (DI, DC)),
            convB=din(p + 'convB', (DI, 1)),
            xprWT=din(p + 'xprWT', (DI, DTR + 2 * DS)),
            dtWT=din(p + 'dtWT', (DTR, DI)),
            dtB=din(p + 'dtB', (DI, 1)),
            Dv=din(p + 'Dv', (DI, 1)),
            outWT=din(p + 'outWT', (DI, D)),
        )
    tmw = mamba_ins('tm_')
    smw = mamba_ins('sm_')

    msgWr = din('msgWr', (D, D))             # msg_w.T
    U1r = din('U1r', (D, D))                 # upd_w[:, :D].T
    U2r = din('U2r', (D, D))                 # upd_w[:, D:].T
    updb = din('updb', (1, D))               # upd_b + upd_w[:,D:] @ msg_b
    gng = din('gng', (1, D))                 # gp norm_g
    gnb = din('gnb', (1, D))                 # gp norm_b
    W1T = din('W1T', (2 * D, D))             # fusion w1.T
    b1 = din('b1', (D, 1))
    w2d = din('w2d', (D, 1))                 # w2[0]-w2[1]
    b2d = din('b2d', (1, 1))                 # b2[0]-b2[1]
    fng = din('fng', (1, D))
    fnb = din('fnb', (1, D))

    out_t = nc.dram_tensor('out', (TOKF, D), F32, kind='ExternalOutput')

    with tile.TileContext(nc) as tc:
        import contextlib
        ctx = contextlib.ExitStack()
        wp = ctx.enter_context(tc.tile_pool(name='wp', bufs=1))
        pp = ctx.enter_context(tc.tile_pool(name='pp', bufs=1))
        gp_ = ctx.enter_context(tc.tile_pool(name='gp', bufs=2))
        sc = ctx.enter_context(tc.tile_pool(name='sc', bufs=1))
        ps = ctx.enter_context(tc.tile_pool(name='ps', bufs=8, space='PSUM'))

        ident = wp.tile([128, 128], F32)
        make_identity(nc, ident)
        ones_row = wp.tile([1, 128], F32)
        nc.vector.memset(ones_row, 1.0)

        def load_w(t, shape):
            """Load dram weight. Rows>128 become [128, R//128, C] (row-block k
            at [:, k, :])."""
            if shape[0] > 128:
                assert shape[0] % 128 == 0 and len(shape) == 2
                nk = shape[0] // 128
                s = wp.tile([128, nk, shape[1]], F32, tag=t.name)
                nc.sync.dma_start(
                    out=s, in_=t.ap().rearrange('(k p) c -> p k c', p=128))
            else:
                s = wp.tile(list(shape), F32, tag=t.name)
                nc.sync.dma_start(out=s, in_=t.ap())
            return s

        def bcast_row(t, L):
            """(1,L) dram vec -> [128, L] sbuf replicated."""
            s = wp.tile([128, L], F32, tag=t.name + '_bc')
            src = t.ap()
            ap = bass.AP(tensor=src.tensor, offset=src.offset,
                         ap=[[0, 128]] + src.ap[1:])
            nc.sync.dma_start(out=s, in_=ap)
            return s

        # weights in sbuf
        W = {}
        for nm, tens in [('adjT', adjT), ('msgWr', msgWr), ('U1r', U1r),
                         ('U2r', U2r), ('W1T', W1T), ('b1', b1),
                         ('w2d', w2d), ('b2d', b2d), ('thf', thf)]:
            W[nm] = load_w(tens, tens.shape)
        for pfx, mw in [('tm', tmw), ('sm', smw)]:
            for nm, tens in mw.items():
                W[pfx + nm] = load_w(tens, tens.shape)
        W['updb'] = bcast_row(updb, D)
        W['gng'] = bcast_row(gng, D)
        W['gnb'] = bcast_row(gnb, D)
        W['fng'] = bcast_row(fng, D)
        W['fnb'] = bcast_row(fnb, D)
        epsc = wp.tile([128, 1], F32)
        nc.vector.memset(epsc, EPS)

        def ln_norm(dst, src_raw, n_par):
            """dst = (src - mean)/sqrt(var+eps) over free dim D. [p, D] tiles."""
            st = gp_.tile([128, 6], F32, tag='ln_st')
            mv = gp_.tile([128, 2], F32, tag='ln_mv')
            nc.vector.bn_stats(out=st[:n_par], in_=src_raw)
            nc.vector.bn_aggr(out=mv[:n_par], in_=st[:n_par])
            nc.scalar.activation(out=mv[:n_par, 1:2], in_=mv[:n_par, 1:2],
                                 func=AF.Sqrt, bias=epsc[:n_par], scale=1.0)
            nc.vector.reciprocal(out=mv[:n_par, 1:2], in_=mv[:n_par, 1:2])
            nc.vector.tensor_scalar(out=dst, in0=src_raw,
                                    scalar1=mv[:n_par, 0:1],
                                    scalar2=mv[:n_par, 1:2],
                                    op0=AL.subtract, op1=AL.mult)

        def transpose128(dst, src, n_par, n_free):
            """dst[f, p] = src[p, f] via PE. src sbuf [n_par, n_free]."""
            pt = ps.tile([128, 128], F32, tag='tp_ps')
            nc.tensor.transpose(pt[:n_free, :n_par], src, ident[:n_par, :n_par])
            nc.scalar.copy(out=dst, in_=pt[:n_free, :n_par])

        # ============== temporal stream ==============
        xtT = pp.tile([128, TOKT], F32, tag='xtT')   # temporal stream output^T

        def mamba_group(pfx, xlnT_ap, xT_res_ap, dstT_ap, gtok, nseq, Lpad, Lval,
                        conv_pad_tag, conv_bufs=2):
            """One seq-group through the mamba inner. gtok = nseq*Lpad tokens.
            xlnT_ap: [128, gtok] LN'd input^T (pre-gamma-folded).
            xT_res_ap: [128, gtok] raw input^T for residual.
            dstT_ap: [128, gtok] output^T destination.
            Lval: valid steps per seq (scan/compute width per seq)."""
            # in_proj -> psum blocks [128, gtok] x4; copy out immediately
            xinp = sc.tile([128, 2, nseq, DC - 1 + Lpad], F32, tag=conv_pad_tag,
                           bufs=conv_bufs)
            zt = gp_.tile([128, 2, gtok], F32, tag='m_z')
            for eb in range(4):
                pt = ps.tile([128, gtok], F32, tag='mm_xz')
                nc.tensor.matmul(pt, W[pfx + 'inWT'][:, eb * 128:(eb + 1) * 128],
                                 xlnT_ap, start=True, stop=True)
                bias_ap = W[pfx + 'inB'][:, eb, :]
                if eb < 2:
                    nc.vector.memset(xinp[:, eb, :, 0:DC - 1], 0.0)
                    nc.scalar.activation(out=xinp[:, eb, :, DC - 1:], in_=pt,
                                         func=AF.Identity, bias=bias_ap, scale=1.0)
                else:
                    nc.scalar.activation(out=zt[:, eb - 2], in_=pt,
                                         func=AF.Identity, bias=bias_ap, scale=1.0)
            # conv (depthwise, 4 taps) + silu -> u
            ut = gp_.tile([128, 2, gtok], F32, tag='m_u')
            cacc = gp_.tile([128, 2, gtok], F32, tag='m_cacc')
            for tau in range(2):
                cw = W[pfx + 'convW'][:, tau, :]
                src = xinp[:, tau]
                nc.vector.tensor_scalar_mul(
                    out=cacc[:, tau].rearrange('p (s l) -> p s l', s=nseq),
                    in0=src[:, :, 0:Lpad], scalar1=cw[:, 0:1])
                for j in range(1, DC):
                    nc.vector.scalar_tensor_tensor(
                        out=cacc[:, tau].rearrange('p (s l) -> p s l', s=nseq),
                        in0=src[:, :, j:j + Lpad], scalar=cw[:, j:j + 1],
                        in1=cacc[:, tau].rearrange('p (s l) -> p s l', s=nseq),
                        op0=AL.mult, op1=AL.add)
                nc.scalar.activation(out=ut[:, tau], in_=cacc[:, tau],
                                     func=AF.Silu,
                                     bias=W[pfx + 'convB'][:, tau, :],
                                     scale=1.0)
            # x_proj: [40, gtok]
            xdb_p = ps.tile([128, gtok], F32, tag='mm_xdb')
            for tau in range(2):
                nc.tensor.matmul(xdb_p[:DTR + 2 * DS],
                                 W[pfx + 'xprWT'][:, tau, :],
                                 ut[:, tau], start=(tau == 0), stop=(tau == 1))
            xdb = gp_.tile([128, gtok], F32, tag='m_xdb')
            nc.scalar.copy(out=xdb[:DTR + 2 * DS], in_=xdb_p[:DTR + 2 * DS])
            # dt = softplus(dtWT.T @ dtr + dtB)
            dt = gp_.tile([128, 2, gtok], F32, tag='m_dt')
            for tau in range(2):
                dtp = ps.tile([128, gtok], F32, tag='mm_dtp')
                nc.tensor.matmul(dtp, W[pfx + 'dtWT'][:, tau * 128:(tau + 1) * 128],
                                 xdb[:DTR], start=True, stop=True)
                nc.scalar.activation(out=dt[:, tau], in_=dtp, func=AF.Softplus,
                                     bias=W[pfx + 'dtB'][:, tau, :],
                                     scale=1.0)
            # g = dt * u
            gdt = gp_.tile([128, 2, gtok], F32, tag='m_g')
            nc.vector.tensor_mul(out=gdt, in0=dt, in1=ut)
            # scan per di-tile. B/C rows are partition-broadcast into a
            # full-width sbuf tile by 32 small DMAs (DMA engines are idle),
            # so the X / h*C products are single wide DVE ops.
            nsl = nseq * Lpad
            bcb = sc.tile([128, 2 * DS, nsl], F32, tag='m_bcb', bufs=1)
            for c in range(2 * DS):
                srow = xdb[DTR + c:DTR + c + 1, :]
                nc.sync.dma_start(
                    out=bcb[:, c, :],
                    in_=bass.AP(tensor=srow.tensor, offset=srow.offset,
                                ap=[[0, 128]] + list(srow.ap)[1:]))
            yv = gp_.tile([128, 2, gtok], F32, tag='m_y')
            import os as _os
            if _os.environ.get('ABL') == 'scan':
                nc.vector.memset(yv, 0.1)
            for tau in range(0 if _os.environ.get('ABL') == 'scan' else 2):
                dA = sc.tile([128, DS, nsl], F32, tag='m_dA')
                Xb = sc.tile([128, DS, nsl], F32, tag='m_X')
                hb = sc.tile([128, DS, nsl], F32, tag='m_h')
                # dA[0] = exp(-dt); zero t=0 of each seq
                nc.scalar.activation(out=dA[:, 0], in_=dt[:, tau],
                                     func=AF.Exp, scale=-1.0)
                nc.vector.memset(
                    dA[:, 0].rearrange('p (s l) -> p s l', s=nseq)[:, :, 0:1], 0.0)
                for n in range(1, DS):
                    nc.vector.tensor_mul(out=dA[:, n], in0=dA[:, n - 1],
                                         in1=dA[:, 0])
                gsl = gdt[:, tau]
                gbc = bass.AP(tensor=gsl.tensor, offset=gsl.offset,
                              ap=[gsl.ap[0], [0, DS], [1, nsl]])
                nc.vector.tensor_tensor(out=Xb, in0=bcb[:, 0:DS, :], in1=gbc,
                                        op=AL.mult)
                nc.vector.tensor_tensor_scan(
                    out=hb.rearrange('p a b -> p (a b)'),
                    data0=dA.rearrange('p a b -> p (a b)'),
                    data1=Xb.rearrange('p a b -> p (a b)'),
                    initial=0.0, op0=AL.mult, op1=AL.add)
                # y = sum_n h*C ; reuse Xb as product buffer
                nc.vector.tensor_tensor(out=Xb, in0=hb, in1=bcb[:, DS:, :],
                                        op=AL.mult)
                nc.vector.tensor_reduce(
                    out=yv[:, tau], in_=Xb.rearrange('p a b -> p b a'),
                    axis=AX.X, op=AL.add)
                # y += u * D
                nc.vector.scalar_tensor_tensor(
                    out=yv[:, tau], in0=ut[:, tau],
                    scalar=W[pfx + 'Dv'][:, tau, :],
                    in1=yv[:, tau], op0=AL.mult, op1=AL.add)
                # y *= silu(z)
                nc.scalar.activation(out=zt[:, tau], in_=zt[:, tau], func=AF.Silu)
                nc.vector.tensor_mul(out=yv[:, tau], in0=yv[:, tau],
                                     in1=zt[:, tau])
            # out_proj + residual
            op = ps.tile([128, gtok], F32, tag='mm_op')
            for tau in range(2):
                nc.tensor.matmul(op, W[pfx + 'outWT'][:, tau, :],
                                 yv[:, tau], start=(tau == 0), stop=(tau == 1))
            nc.vector.scalar_tensor_tensor(out=dstT_ap, in0=op, scalar=1.0,
                                           in1=xT_res_ap, op0=AL.mult, op1=AL.add)

        for g in range(NGT):
            c0 = g * FT
            xTg = gp_.tile([128, FT], F32, tag='t_xTg')
            xlnTg = gp_.tile([128, FT], F32, tag='t_xlnTg')
            for sub in range(2):
                r0 = c0 + sub * 128
                nr = min(128, FT - sub * 128)
                xr = gp_.tile([128, D], F32, tag='t_xr')
                nc.sync.dma_start(out=xr[:nr], in_=x_nt.ap()[r0:r0 + nr])
                xl = gp_.tile([128, D], F32, tag='t_xl')
                ln_norm(xl[:nr], xr[:nr], nr)
                transpose128(xTg[:, sub * 128:sub * 128 + nr], xr[:nr], nr, D)
                transpose128(xlnTg[:, sub * 128:sub * 128 + nr], xl[:nr], nr, D)
            mamba_group('tm', xlnTg, xTg, xtT[:, c0:c0 + FT], FT, GT, T, T,
                        'convp_t')

        # ============== spatial stream ==============
        xspT = pp.tile([128, TH, NP2], F32, tag='xspT')   # spatial mamba out^T (padded n)
        for t2 in range(TH):
            # load raw [NP2, D] (2 tiles), transpose
            xr = [gp_.tile([128, D], F32, tag='s_xr' + str(h), name='s_xr' + str(h))
                  for h in range(2)]
            for h in range(2):
                nc.sync.dma_start(out=xr[h], in_=x_sp.ap()[t2, h * 128:(h + 1) * 128])
            xsT = gp_.tile([128, NP2], F32, tag='s_xsT')
            for h in range(2):
                transpose128(xsT[:, h * 128:(h + 1) * 128], xr[h], 128, D)
            # msg = x @ msgW.T  [tok, d] per half
            msg = gp_.tile([128, 2, D], F32, tag='s_msg')
            for h in range(2):
                pt = ps.tile([128, D], F32, tag='mm_msg')
                nc.tensor.matmul(pt, xsT[:, h * 128:(h + 1) * 128], W['msgWr'],
                                 start=True, stop=True)
                nc.scalar.copy(out=msg[:, h], in_=pt)
            # agg[nb] = sum_m adjT[m, nb].T... out [n-block, d]
            aggT = gp_.tile([128, NP2], F32, tag='s_aggT')
            for nb in range(2):
                pt = ps.tile([128, D], F32, tag='mm_agg')
                for km in range(2):
                    nc.tensor.matmul(pt,
                                     W['adjT'][:, km, nb * 128:(nb + 1) * 128],
                                     msg[:, km], start=(km == 0), stop=(km == 1))
                ag = gp_.tile([128, D], F32, tag='s_ag')
                nc.scalar.copy(out=ag, in_=pt)
                transpose128(aggT[:, nb * 128:(nb + 1) * 128], ag, 128, D)
            # upd + residual + graph LN -> xg [tok, d]
            xgT = gp_.tile([128, NP2], F32, tag='s_xgT')    # LN_graph(xg)^T pre-sm-LN
            xg_td = [gp_.tile([128, D], F32, tag='s_xg' + str(h), name='s_xg' + str(h))
                     for h in range(2)]
            for h in range(2):
                pt = ps.tile([128, D], F32, tag='mm_upd')
                nc.tensor.matmul(pt, xsT[:, h * 128:(h + 1) * 128], W['U1r'],
                                 start=True, stop=False)
                nc.tensor.matmul(pt, aggT[:, h * 128:(h + 1) * 128], W['U2r'],
                                 start=False, stop=True)
                up = gp_.tile([128, D], F32, tag='s_up')
                nc.vector.scalar_tensor_tensor(out=up, in0=pt, scalar=1.0,
                                               in1=xr[h], op0=AL.mult, op1=AL.add)
                nc.vector.tensor_add(out=up, in0=up, in1=W['updb'])
                ln_norm(up, up, 128)
                nc.vector.tensor_mul(out=up, in0=up, in1=W['gng'])
                nc.vector.tensor_add(out=up, in0=up, in1=W['gnb'])
                nc.vector.tensor_copy(out=xg_td[h], in_=up)
            # sm pre-LN (gamma folded into sm inWT)
            xgl = gp_.tile([128, D], F32, tag='s_xgl')
            for h in range(2):
                ln_norm(xgl, xg_td[h], 128)
                transpose128(xgT[:, h * 128:(h + 1) * 128], xgl, 128, D)
            # residual^T for the mamba block = xg^T
            xgrT = gp_.tile([128, NP2], F32, tag='s_xgrT')
            for h in range(2):
                transpose128(xgrT[:, h * 128:(h + 1) * 128], xg_td[h], 128, D)
            mamba_group('sm', xgT, xgrT, xspT[:, t2], NP2, 1, NP2, N, 'convp_s',
                        conv_bufs=1)

        # ============== fusion ==============
        # xt half-select: view xtT cols as (t2, n): col = n*T + t0 + t2
        xtv = xtT.rearrange('p (n t) -> p t n', n=N)   # [128, T, N]
        xsel = pp.tile([128, TOKF], F32, tag='xsel')
        xsel3 = xsel.rearrange('p (a b) -> p a b', a=TH)
        nc.vector.tensor_tensor(out=xsel3, in0=xtv[:, TH:, :],
                                in1=xtv[:, :TH, :], op=AL.subtract)
        nc.vector.scalar_tensor_tensor(
            out=xsel3, in0=xsel3, scalar=W['thf'], in1=xtv[:, :TH, :],
            op0=AL.mult, op1=AL.add)
        # xsp valid view -> contiguous
        xspv = pp.tile([128, TOKF], F32, tag='xspv')
        nc.vector.tensor_copy(
            out=xspv.rearrange('p (a b) -> p a b', a=TH),
            in_=xspT[:, :, 0:N])
        # h1 = gelu(W1 @ comb + b1)
        h1 = pp.tile([128, TOKF], F32, tag='xtT', name='h1')
        nck = (TOKF + 511) // 512
        for ic in range(nck):
            c0 = ic * 512
            cw_ = min(512, TOKF - c0)
            pt = ps.tile([128, 512], F32, tag='mm_h1')
            nc.tensor.matmul(pt[:, :cw_], W['W1T'][:, 0, :], xsel[:, c0:c0 + cw_],
                             start=True, stop=False)
            nc.tensor.matmul(pt[:, :cw_], W['W1T'][:, 1, :], xspv[:, c0:c0 + cw_],
                             start=False, stop=True)
            nc.scalar.activation(out=h1[:, c0:c0 + cw_], in_=pt[:, :cw_],
                                 func=AF.Gelu, bias=W['b1'], scale=1.0)
        # gate logit diff -> sigmoid
        g0r = pp.tile([1, TOKF], F32, tag='xspT', name='g0r')
        for ic in range(nck):
            c0 = ic * 512
            cw_ = min(512, TOKF - c0)
            pt = ps.tile([1, 512], F32, tag='mm_g0')
            nc.tensor.matmul(pt[:, :cw_], W['w2d'], h1[:, c0:c0 + cw_],
                             start=True, stop=True)
            nc.scalar.activation(out=g0r[:, c0:c0 + cw_], in_=pt[:, :cw_],
                                 func=AF.Sigmoid, bias=W['b2d'], scale=1.0)
        # g0 column form [128, 20]
        ntf = (TOKF + 127) // 128
        g0c = gp_.tile([128, ntf], F32, tag='f_g0c')
        for it in range(ntf):
            r0 = it * 128
            nr = min(128, TOKF - r0)
            pt = ps.tile([128, 1], F32, tag='tp_g0')
            nc.tensor.transpose(pt[:nr, 0:1], g0r[:, r0:r0 + nr], ident[0:1, 0:1])
            nc.scalar.copy(out=g0c[:nr, it:it + 1], in_=pt[:nr, 0:1])
        # back to [tok, d], blend, final LN
        for it in range(ntf):
            r0 = it * 128
            nr = min(128, TOKF - r0)
            xtd = gp_.tile([128, D], F32, tag='f_xtd')
            xsd = gp_.tile([128, D], F32, tag='f_xsd')
            transpose128(xtd[:nr], xsel[:, r0:r0 + nr], 128, nr)
            transpose128(xsd[:nr], xspv[:, r0:r0 + nr], 128, nr)
            dd = gp_.tile([128, D], F32, tag='f_dd')
            nc.vector.tensor_sub(out=dd[:nr], in0=xtd[:nr], in1=xsd[:nr])
            fu = gp_.tile([128, D], F32, tag='f_fu')
            nc.vector.scalar_tensor_tensor(out=fu[:nr], in0=dd[:nr],
                                           scalar=g0c[:nr, it:it + 1],
                                           in1=xsd[:nr], op0=AL.mult, op1=AL.add)
            ln_norm(fu[:nr], fu[:nr], nr)
            nc.vector.tensor_mul(out=fu[:nr], in0=fu[:nr], in1=W['fng'][:nr])
            nc.vector.tensor_add(out=fu[:nr], in0=fu[:nr], in1=W['fnb'][:nr])
            nc.sync.dma_start(out=out_t.ap()[r0:r0 + nr], in_=fu[:nr])

        ctx.close()
    nc.compile()
    return nc


def _prep_inputs(x, adj, tm, sm, gp, fp):
    """Host-side weight prep; returns (shared weight map, per-core input maps)."""
    f32 = np.float32

    def mamba_prep(p, w):
        A = -np.exp(np.asarray(w['A_log'], f32))
        expect = -np.broadcast_to(np.arange(1, DS + 1, dtype=f32), (DI, DS))
        assert np.allclose(A, expect, rtol=1e-5, atol=1e-5), \
            'A_log structure changed; scan power-chain invalid'
        g = np.asarray(w['norm_g'], f32)
        b = np.asarray(w['norm_b'], f32)
        iw = np.asarray(w['in_proj_w'], f32)
        return {
            p + 'inWT': np.ascontiguousarray((iw * g[None, :]).T),
            p + 'inB': (iw @ b).reshape(2 * DI, 1),
            p + 'convW': np.asarray(w['conv_w'], f32).reshape(DI, DC),
            p + 'convB': np.asarray(w['conv_b'], f32).reshape(DI, 1),
            p + 'xprWT': np.ascontiguousarray(np.asarray(w['x_proj_w'], f32).T),
            p + 'dtWT': np.ascontiguousarray(np.asarray(w['dt_proj_w'], f32).T),
            p + 'dtB': np.asarray(w['dt_proj_b'], f32).reshape(DI, 1),
            p + 'Dv': np.asarray(w['D'], f32).reshape(DI, 1),
            p + 'outWT': np.ascontiguousarray(np.asarray(w['out_proj_w'], f32).T),
        }

    shared = {}
    shared.update(mamba_prep('tm_', tm))
    shared.update(mamba_prep('sm_', sm))
    adjT = np.zeros((NP2, NP2), f32)
    adjT[:N, :N] = np.asarray(adj, f32).T
    shared['adjT'] = adjT
    uw = np.asarray(gp['upd_w'], f32)
    shared['msgWr'] = np.ascontiguousarray(np.asarray(gp['msg_w'], f32).T)
    shared['U1r'] = np.ascontiguousarray(uw[:, :D].T)
    shared['U2r'] = np.ascontiguousarray(uw[:, D:].T)
    shared['updb'] = (np.asarray(gp['upd_b'], f32)
                      + uw[:, D:] @ np.asarray(gp['msg_b'], f32)).reshape(1, D)
    shared['gng'] = np.asarray(gp['norm_g'], f32).reshape(1, D)
    shared['gnb'] = np.asarray(gp['norm_b'], f32).reshape(1, D)
    w1 = np.asarray(fp['w1'], f32)
    w2 = np.asarray(fp['w2'], f32)
    b2 = np.asarray(fp['b2'], f32)
    shared['W1T'] = np.ascontiguousarray(w1.T)
    shared['b1'] = np.asarray(fp['b1'], f32).reshape(D, 1)
    shared['w2d'] = (w2[0] - w2[1]).reshape(D, 1)
    shared['b2d'] = np.array([[b2[0] - b2[1]]], f32)
    shared['fng'] = np.asarray(fp['norm_g'], f32).reshape(1, D)
    shared['fnb'] = np.asarray(fp['norm_b'], f32).reshape(1, D)

    x = np.asarray(x, f32)
    in_maps = []
    for k in range(8):
        b_, th = k // 2, k % 2
        m = dict(shared)
        m['x_nt'] = np.ascontiguousarray(
            x[b_].transpose(1, 0, 2)).reshape(TOKT, D)
        xsp = np.zeros((TH, NP2, D), f32)
        xsp[:, :N] = x[b_, th * TH:(th + 1) * TH]
        m['x_sp'] = xsp
        m['thf'] = np.full((128, 1), float(th), f32)
        in_maps.append(m)
    return in_maps


def kernel(x, adj, tm, sm, gp, fp):
    from concourse.bass_utils import run_bass_kernel_spmd
    if 'nc' not in _CACHE:
        _CACHE['nc'] = _build()
    nc = _CACHE['nc']
    in_maps = _prep_inputs(x, adj, tm, sm, gp, fp)
    res = run_bass_kernel_spmd(nc, in_maps, core_ids=list(range(8)))
    out = np.empty((B, T, N, D), np.float32)
    for k in range(8):
        b_, th = k // 2, k % 2
        out[b_, th * TH:(th + 1) * TH] = res.results[k]['out'].reshape(TH, N, D)
    return out
